# revision 19
# baseline (speedup 1.0000x reference)
"""Single-head causal attention with RoPE on 8 TRN2 NeuronCores.

Sharding: core c -> batch c//2, parity p = c%2 takes the interleaved
512-row q-blocks {p, p+2, p+4, p+6} of T=4096. Each core projects
q/k/v only for its OWN 2048 rows (bf16 matmuls); pairs exchange bf16
K/V via four quarter-sized AllReduce(add) collectives on DRAM bounce
buffers (fired as each 512-row group finishes projecting, so they
hide under the rest of phase 1) and recover the partner's half as
(sum - own) on the vector engine.

Attention computes transposed scores S^T[s, q]; the AV matmul uses
P^T chunks as the stationary operand with a [v | ones] moving
operand, so each 128-q output chunk lands row-major in PSUM with its
softmax denominator in column 128 - no output transposes and no
separate row-sum matmuls. PSUM accumulation chains sharing a bank
are zeroed once via memset (matmul start=True zeroes the whole bank,
which would wipe the sibling chain). Causal masking: static
triangular mask on the diagonal 128x128 sub-blocks (with the q-range
of diagonal 512-blocks trimmed per s-subtile) plus a data-dependent
exp bias (-1e9) for the partner tail block.
"""
import numpy as np
import ml_dtypes

B, T, C, HD = 4, 4096, 2048, 128
P = 128
BS = 512
T2 = T // 2          # own rows per core
NT = T2 // P         # 16 own 128-blocks
SCALE = float(C) ** -0.5
NEG = -1.0e9
bf16 = ml_dtypes.bfloat16


def build():
    import concourse.bass as bass
    import concourse.mybir as mybir
    import bass_rust
    from concourse.tile import TileContext
    from concourse.masks import make_identity

    f32 = mybir.dt.float32
    bf = mybir.dt.bfloat16
    EXP = mybir.ActivationFunctionType.Exp

    nc = bass.Bass(num_devices=8)
    xt = nc.declare_dram_parameter("xt", [C, T2], bf, isOutput=False)
    w = nc.declare_dram_parameter("w", [C, 3 * HD], bf, isOutput=False)
    cos2 = nc.declare_dram_parameter("cos2", [T2, P], bf, isOutput=False)
    sin2 = nc.declare_dram_parameter("sin2", [T2, P], bf, isOutput=False)
    tailb = nc.declare_dram_parameter("tailb", [P, 1], f32, isOutput=False)
    out = nc.declare_dram_parameter("out", [T2, HD], f32, isOutput=True)

    VSTR = 132           # v block stride in vs ([128 v | 1 ones | 3 pad])
    H = 64

    with TileContext(nc) as tc:
        with (
            tc.tile_pool(name="const", bufs=1) as cp,
            tc.tile_pool(name="xp", bufs=1) as xp,
            tc.tile_pool(name="rot", bufs=2) as rp,
            tc.tile_pool(name="pt", bufs=3) as ptp,
            tc.tile_pool(name="osb", bufs=2) as osb,
            tc.tile_pool(name="rec", bufs=2) as rec,
            tc.tile_pool(name="dram", bufs=1, space="DRAM") as dram,
        ):
            # ---- constants / resident tensors ----
            identf = cp.tile([P, P], f32, tag="identf")
            make_identity(nc, identf[:])
            identb = cp.tile([P, P], bf, tag="identb")
            nc.vector.tensor_copy(identb[:], identf[:])
            trif = cp.tile([P, P], f32, tag="trif")
            nc.gpsimd.memset(trif[:], 0.0)
            # tri[s, q] = 1.0 where s <= q
            nc.gpsimd.affine_select(
                out=trif[:], in_=trif[:],
                compare_op=mybir.AluOpType.is_gt,
                fill=1.0, base=0,
                pattern=[[-1, P]], channel_multiplier=1,
            )
            tri = cp.tile([P, P], bf, tag="tri")
            nc.vector.tensor_copy(tri[:], trif[:])
            # x for tg=0 first: it gates the first projection matmuls
            xbigs = []
            for tg in range(4):
                xbig = xp.tile([P, 16 * BS], bf, tag=f"x{tg}",
                               name=f"x{tg}")
                xbigs.append(xbig)
            nc.sync.dma_start(
                xbigs[0][:].rearrange("p (k n) -> p k n", k=16),
                xt[:, 0:BS].rearrange("(k p) n -> p k n", p=P))
            wt = cp.tile([P, 16 * 384], bf, tag="wt")
            for g in range(4):
                nc.sync.dma_start(
                    wt[:, g * 4 * 384:(g + 1) * 4 * 384].rearrange(
                        "p (k n) -> p k n", k=4),
                    w[g * 512:(g + 1) * 512, :].rearrange(
                        "(k p) n -> p k n", p=P))
            cst = cp.tile([P, NT * P], bf, tag="cst")
            snt = cp.tile([P, NT * P], bf, tag="snt")
            for g in range(2):
                sl = slice(g * 8 * P, (g + 1) * 8 * P)
                nc.sync.dma_start(
                    cst[:, sl].rearrange("p (k n) -> p k n", k=8),
                    cos2[g * 8 * P:(g + 1) * 8 * P, :].rearrange(
                        "(k p) n -> p k n", p=P))
                nc.sync.dma_start(
                    snt[:, sl].rearrange("p (k n) -> p k n", k=8),
                    sin2[g * 8 * P:(g + 1) * 8 * P, :].rearrange(
                        "(k p) n -> p k n", p=P))
            tb = cp.tile([P, 1], f32, tag="tb")
            nc.sync.dma_start(tb[:], tailb[:])
            for tg in range(1, 4):
                nc.sync.dma_start(
                    xbigs[tg][:].rearrange("p (k n) -> p k n", k=16),
                    xt[:, tg * BS:(tg + 1) * BS].rearrange(
                        "(k p) n -> p k n", p=P))

            qT = cp.tile([P, NT * P], bf, tag="qT")      # [d, 2048] own q^T
            kT = cp.tile([P, 32 * P], bf, tag="kT")      # [d, 4096] own|partner
            vs = cp.tile([P, 32 * VSTR], bf, tag="vs")   # v rows + ones col
            nc.gpsimd.memset(vs[:], 0.0)
            ones_ap = vs[:].rearrange("p (b c) -> p b c", b=32)[:, :, P:P + 1]
            nc.gpsimd.memset(ones_ap, 1.0)

            # DRAM bounce per half: [k blocks 8h..8h+7 | v blocks ...]
            cin = [dram.tile([P, 16 * P], bf, name=f"cin{q}")
                   for q in range(2)]
            cout = [dram.tile([P, 16 * P], bf, name=f"cout{q}")
                    for q in range(2)]

            # ---- phase 1: projection + RoPE (own 2048 rows) ----
            with tc.tile_pool(name="pps", bufs=2, space="PSUM") as pps, \
                 tc.tile_pool(name="tps", bufs=2, space="PSUM") as tps:
                for tg in range(4):
                    xbig = xbigs[tg]
                    for sub in range(4):
                        t128 = tg * 4 + sub
                        pp = pps.tile([P, 384], f32, tag="pp")
                        for ci in range(16):
                            nc.tensor.matmul(
                                pp[:],
                                xbig[:, ci * BS + sub * P:
                                     ci * BS + (sub + 1) * P],
                                wt[:, ci * 384:(ci + 1) * 384],
                                start=(ci == 0), stop=(ci == 15))
                        cs = cst[:, t128 * P:(t128 + 1) * P]
                        sn = snt[:, t128 * P:(t128 + 1) * P]

                        def rope(src_off, dst):
                            s0 = pp[:, src_off:src_off + P]
                            nc.vector.tensor_mul(dst[:], s0, cs)
                            tmp = rp.tile([P, P], bf, tag="ropetmp")
                            nc.vector.tensor_mul(
                                tmp[:, 0:H], pp[:, src_off + H:src_off + P],
                                sn[:, 0:H])
                            nc.vector.tensor_mul(
                                tmp[:, H:P], pp[:, src_off:src_off + H],
                                sn[:, H:P])
                            nc.vector.tensor_add(dst[:], dst[:], tmp[:])

                        rk = rp.tile([P, P], bf, tag="rk")
                        rope(0, rk)
                        tpk = tps.tile([P, P], bf, tag="tp")
                        nc.tensor.transpose(tpk[:], rk[:], identb[:])
                        nc.vector.tensor_copy(kT[:, t128 * P:(t128 + 1) * P],
                                              tpk[:])
                        nc.scalar.copy(vs[:, t128 * VSTR:t128 * VSTR + P],
                                       pp[:, P:2 * P])
                        rq = rp.tile([P, P], bf, tag="rq")
                        rope(2 * P, rq)
                        tpq = tps.tile([P, P], bf, tag="tp")
                        nc.tensor.transpose(tpq[:], rq[:], identb[:])
                        nc.vector.tensor_copy(qT[:, t128 * P:(t128 + 1) * P],
                                              tpq[:])

                    # half exchange after tg 1 and 3: k/v blocks 8h..8h+7.
                    # All exchange DMAs ride the otherwise-idle gpsimd queue
                    # so their waits never block the x/out DMA issue stream.
                    if tg % 2 == 1:
                        h = tg // 2
                        nc.gpsimd.dma_start(
                            cin[h][:, 0:8 * P],
                            kT[:, h * 8 * P:(h + 1) * 8 * P])
                        vsrc = vs[:, h * 8 * VSTR:(h + 1) * 8 * VSTR
                                  ].rearrange("p (b c) -> p b c",
                                              b=8)[:, :, 0:P]
                        nc.gpsimd.dma_start(
                            cin[h][:, 8 * P:16 * P].rearrange(
                                "p (b c) -> p b c", b=8), vsrc)
                        nc.gpsimd.collective_compute(
                            "AllReduce", mybir.AluOpType.add,
                            replica_groups=[[0, 1], [2, 3], [4, 5], [6, 7]],
                            ins=[cin[h].opt()], outs=[cout[h].opt()],
                        )
                        ksum = rec.tile([P, 8 * P], bf, tag="ksum")
                        vsum = rec.tile([P, 8 * P], bf, tag="vsum")
                        nc.gpsimd.dma_start(ksum[:], cout[h][:, 0:8 * P])
                        nc.gpsimd.dma_start(vsum[:], cout[h][:, 8 * P:16 * P])
                        nc.vector.tensor_sub(
                            kT[:, (16 + 8 * h) * P:(24 + 8 * h) * P],
                            ksum[:], kT[:, h * 8 * P:(h + 1) * 8 * P])
                        for i in range(8):
                            blk = 16 + 8 * h + i
                            own = 8 * h + i
                            nc.vector.tensor_sub(
                                vs[:, blk * VSTR:blk * VSTR + P],
                                vsum[:, i * P:(i + 1) * P],
                                vs[:, own * VSTR:own * VSTR + P])

            # ---- phase 2: attention per q-slot ----
            with tc.tile_pool(name="sps", bufs=2, space="PSUM") as sps, \
                 tc.tile_pool(name="ops", bufs=2, space="PSUM") as ops:
                for j in range(4):
                    o = [ops.tile([P, 2 * 129], f32, tag=f"o{m}",
                                  name=f"o{m}") for m in range(2)]
                    nc.vector.memset(o[0][:], 0.0)
                    nc.vector.memset(o[1][:], 0.0)
                    nav = [0] * 4
                    tot = [8 * j + qc + 5 for qc in range(4)]
                    slots = ([("own", si, "full") for si in range(j)]
                             + [("own", j, "diag")]
                             + [("part", pi, "full") for pi in range(j)]
                             + [("part", j, "tail")])
                    for (side, si, kind) in slots:
                        base = si * 4 if side == "own" else 16 + si * 4
                        for sp in range(2):      # pairs (st, st+1)
                            sts = (2 * sp, 2 * sp + 1)
                            Sps = sps.tile([P, 2 * BS], f32, tag="S")
                            Pt = ptp.tile([P, 2 * BS], bf, tag="Pt")
                            offs = []
                            off = 0
                            for st in sts:
                                blk = base + st
                                trim = st * P if kind == "diag" else 0
                                qlen = BS - trim
                                nc.tensor.matmul(
                                    Sps[:, off:off + qlen],
                                    kT[:, blk * P:(blk + 1) * P],
                                    qT[:, j * BS + trim:j * BS + BS],
                                    start=True, stop=True)
                                offs.append((st, blk, trim, qlen, off))
                                off += qlen
                            bias = tb[:, 0:1] if kind == "tail" else 0.0
                            nc.scalar.activation(Pt[:, 0:off], Sps[:, 0:off],
                                                 EXP, bias=bias, scale=SCALE)
                            for (st, blk, trim, qlen, o0) in offs:
                                if kind == "diag":
                                    nc.vector.tensor_mul(
                                        Pt[:, o0:o0 + P], Pt[:, o0:o0 + P],
                                        tri[:])
                                for ch in range(qlen // P):
                                    qc = trim // P + ch
                                    om, oc = o[qc // 2], (qc % 2) * 129
                                    nc.tensor.matmul(
                                        om[:, oc:oc + 129],
                                        Pt[:, o0 + ch * P:o0 + (ch + 1) * P],
                                        vs[:, blk * VSTR:blk * VSTR + 129],
                                        start=False,
                                        stop=(nav[qc] == tot[qc] - 1),
                                        skip_group_check=True)
                                    nav[qc] += 1
                    # normalize + store
                    obig = osb.tile([P, 4 * P], f32, tag="obig")
                    for qc in range(4):
                        om, oc = o[qc // 2], (qc % 2) * 129
                        rcp = osb.tile([P, 1], f32, tag="rcp")
                        nc.vector.reciprocal(rcp[:], om[:, oc + P:oc + P + 1])
                        nc.vector.tensor_scalar_mul(
                            obig[:, qc * P:(qc + 1) * P], om[:, oc:oc + P],
                            rcp[:])
                    nc.sync.dma_start(
                        out[j * BS:(j + 1) * BS, :].rearrange(
                            "(k p) n -> p k n", p=P),
                        obig[:].rearrange("p (k n) -> p k n", k=4))

    bass_rust.generate_event_semaphores(nc)
    return nc


_CACHE = {}


def _get_nc():
    if "nc" not in _CACHE:
        _CACHE["nc"] = build()
    return _CACHE["nc"]


def _prep_inputs(x, Wq, Wk, Wv, cos, sin):
    perm = np.concatenate([np.arange(0, HD, 2), np.arange(1, HD, 2)])
    wq = Wq[perm].astype(np.float32)
    wk = Wk[perm].astype(np.float32)
    w = np.concatenate([wk.T, Wv.T.astype(np.float32), wq.T], axis=1)
    w = np.ascontiguousarray(w).astype(bf16)   # [C, 384] = [k|v|q]
    cos2 = np.concatenate([cos, cos], axis=1).astype(np.float32)
    sin2 = np.concatenate([-sin, sin], axis=1).astype(np.float32)
    in_maps = []
    for c in range(8):
        b, par = c // 2, c % 2
        own = np.concatenate(
            [np.arange(a * BS, (a + 1) * BS) for a in (par, par + 2,
                                                       par + 4, par + 6)])
        xb = np.asarray(x[b], np.float32)
        xtp = np.ascontiguousarray(xb[own].T).astype(bf16)      # [C, T2]
        c2 = np.ascontiguousarray(cos2[own]).astype(bf16)
        s2 = np.ascontiguousarray(sin2[own]).astype(bf16)
        tb = np.full((P, 1), NEG if par == 0 else 0.0, np.float32)
        in_maps.append({"xt": xtp, "w": w, "cos2": c2, "sin2": s2,
                        "tailb": tb})
    return in_maps


def _run(x, Wq, Wk, Wv, cos, sin, trace=False):
    from concourse.bass_utils import run_bass_kernel_spmd
    nc = _get_nc()
    in_maps = _prep_inputs(x, Wq, Wk, Wv, cos, sin)
    res = run_bass_kernel_spmd(nc, in_maps, list(range(8)), trace=trace)
    full = np.empty((B, T, HD), np.float32)
    for c in range(8):
        b, par = c // 2, c % 2
        oc = res.results[c]["out"]
        for j in range(4):
            ab = par + 2 * j
            full[b, ab * BS:(ab + 1) * BS] = oc[j * BS:(j + 1) * BS]
    return full, res


def kernel(x, Wq, Wk, Wv, cos, sin):
    return _run(x, Wq, Wk, Wv, cos, sin, trace=False)[0]


# revision 21
# speedup vs baseline: 1.0489x; 1.0489x over previous
"""Single-head causal attention with RoPE on 8 TRN2 NeuronCores.

Sharding: core c -> batch c//2, parity p = c%2 takes the interleaved
512-row q-blocks {p, p+2, p+4, p+6} of T=4096. Each core projects
q/k/v only for its OWN 2048 rows (bf16 matmuls); pairs exchange bf16
K/V via four quarter-sized AllReduce(add) collectives on DRAM bounce
buffers (fired as each 512-row group finishes projecting, so they
hide under the rest of phase 1) and recover the partner's half as
(sum - own) on the vector engine.

Attention computes transposed scores S^T[s, q]; the AV matmul uses
P^T chunks as the stationary operand with a [v | ones] moving
operand, so each 128-q output chunk lands row-major in PSUM with its
softmax denominator in column 128 - no output transposes and no
separate row-sum matmuls. PSUM accumulation chains sharing a bank
are zeroed once via memset (matmul start=True zeroes the whole bank,
which would wipe the sibling chain). Causal masking: static
triangular mask on the diagonal 128x128 sub-blocks (with the q-range
of diagonal 512-blocks trimmed per s-subtile) plus a data-dependent
exp bias (-1e9) for the partner tail block.
"""
import numpy as np
import ml_dtypes

B, T, C, HD = 4, 4096, 2048, 128
P = 128
BS = 512
T2 = T // 2          # own rows per core
NT = T2 // P         # 16 own 128-blocks
SCALE = float(C) ** -0.5
NEG = -1.0e9
bf16 = ml_dtypes.bfloat16


def build():
    import concourse.bass as bass
    import concourse.mybir as mybir
    import bass_rust
    from concourse.tile import TileContext
    from concourse.masks import make_identity

    f32 = mybir.dt.float32
    bf = mybir.dt.bfloat16
    EXP = mybir.ActivationFunctionType.Exp

    nc = bass.Bass(num_devices=8)
    xt = nc.declare_dram_parameter("xt", [C, T2], bf, isOutput=False)
    w = nc.declare_dram_parameter("w", [C, 3 * HD], bf, isOutput=False)
    cos2 = nc.declare_dram_parameter("cos2", [T2, P], bf, isOutput=False)
    sin2 = nc.declare_dram_parameter("sin2", [T2, P], bf, isOutput=False)
    tailb = nc.declare_dram_parameter("tailb", [P, 1], f32, isOutput=False)
    out = nc.declare_dram_parameter("out", [T2, HD], f32, isOutput=True)

    VSTR = 132           # v block stride in vs ([128 v | 1 ones | 3 pad])
    H = 64

    with TileContext(nc) as tc:
        with (
            tc.tile_pool(name="const", bufs=1) as cp,
            tc.tile_pool(name="xp", bufs=1) as xp,
            tc.tile_pool(name="rot", bufs=2) as rp,
            tc.tile_pool(name="pt", bufs=3) as ptp,
            tc.tile_pool(name="osb", bufs=2) as osb,
            tc.tile_pool(name="rec", bufs=2) as rec,
            tc.tile_pool(name="dram", bufs=1, space="DRAM") as dram,
        ):
            # ---- constants / resident tensors ----
            identf = cp.tile([P, P], f32, tag="identf")
            make_identity(nc, identf[:])
            identb = cp.tile([P, P], bf, tag="identb")
            nc.vector.tensor_copy(identb[:], identf[:])
            trif = cp.tile([P, P], f32, tag="trif")
            nc.gpsimd.memset(trif[:], 0.0)
            # tri[s, q] = 1.0 where s <= q
            nc.gpsimd.affine_select(
                out=trif[:], in_=trif[:],
                compare_op=mybir.AluOpType.is_gt,
                fill=1.0, base=0,
                pattern=[[-1, P]], channel_multiplier=1,
            )
            tri = cp.tile([P, P], bf, tag="tri")
            nc.vector.tensor_copy(tri[:], trif[:])
            # x for tg=0 first: it gates the first projection matmuls
            xbigs = []
            for tg in range(4):
                xbig = xp.tile([P, 16 * BS], bf, tag=f"x{tg}",
                               name=f"x{tg}")
                xbigs.append(xbig)
            nc.sync.dma_start(
                xbigs[0][:].rearrange("p (k n) -> p k n", k=16),
                xt[:, 0:BS].rearrange("(k p) n -> p k n", p=P))
            wt = cp.tile([P, 16 * 384], bf, tag="wt")
            for g in range(4):
                nc.sync.dma_start(
                    wt[:, g * 4 * 384:(g + 1) * 4 * 384].rearrange(
                        "p (k n) -> p k n", k=4),
                    w[g * 512:(g + 1) * 512, :].rearrange(
                        "(k p) n -> p k n", p=P))
            cst = cp.tile([P, NT * P], bf, tag="cst")
            snt = cp.tile([P, NT * P], bf, tag="snt")
            for g in range(2):
                sl = slice(g * 8 * P, (g + 1) * 8 * P)
                nc.sync.dma_start(
                    cst[:, sl].rearrange("p (k n) -> p k n", k=8),
                    cos2[g * 8 * P:(g + 1) * 8 * P, :].rearrange(
                        "(k p) n -> p k n", p=P))
                nc.sync.dma_start(
                    snt[:, sl].rearrange("p (k n) -> p k n", k=8),
                    sin2[g * 8 * P:(g + 1) * 8 * P, :].rearrange(
                        "(k p) n -> p k n", p=P))
            tb = cp.tile([P, 1], f32, tag="tb")
            nc.sync.dma_start(tb[:], tailb[:])
            for tg in range(1, 4):
                nc.sync.dma_start(
                    xbigs[tg][:].rearrange("p (k n) -> p k n", k=16),
                    xt[:, tg * BS:(tg + 1) * BS].rearrange(
                        "(k p) n -> p k n", p=P))

            qT = cp.tile([P, NT * P], bf, tag="qT")      # [d, 2048] own q^T
            kT = cp.tile([P, 32 * P], bf, tag="kT")      # [d, 4096] own|partner
            vs = cp.tile([P, 32 * VSTR], bf, tag="vs")   # v rows + ones col
            nc.gpsimd.memset(vs[:], 0.0)
            ones_ap = vs[:].rearrange("p (b c) -> p b c", b=32)[:, :, P:P + 1]
            nc.gpsimd.memset(ones_ap, 1.0)

            # DRAM bounce per quarter: [k blocks 4q..4q+3 | v blocks ...]
            cin = [dram.tile([P, 8 * P], bf, name=f"cin{q}")
                   for q in range(4)]
            cout = [dram.tile([P, 8 * P], bf, name=f"cout{q}")
                    for q in range(4)]
            # dummy collective: absorbs cross-core launch skew early, so the
            # first real exchange doesn't pay it (contents are irrelevant)
            cind = dram.tile([1, 16], f32, name="cind")
            coutd = dram.tile([1, 16], f32, name="coutd")
            nc.gpsimd.collective_compute(
                "AllReduce", mybir.AluOpType.add,
                replica_groups=[[0, 1], [2, 3], [4, 5], [6, 7]],
                ins=[cind.opt()], outs=[coutd.opt()],
            )

            # ---- phase 1: projection + RoPE (own 2048 rows) ----
            with tc.tile_pool(name="pps", bufs=2, space="PSUM") as pps, \
                 tc.tile_pool(name="tps", bufs=2, space="PSUM") as tps:
                for tg in range(4):
                    xbig = xbigs[tg]
                    for sub in range(4):
                        t128 = tg * 4 + sub
                        pp = pps.tile([P, 384], f32, tag="pp")
                        for ci in range(16):
                            nc.tensor.matmul(
                                pp[:],
                                xbig[:, ci * BS + sub * P:
                                     ci * BS + (sub + 1) * P],
                                wt[:, ci * 384:(ci + 1) * 384],
                                start=(ci == 0), stop=(ci == 15))
                        cs = cst[:, t128 * P:(t128 + 1) * P]
                        sn = snt[:, t128 * P:(t128 + 1) * P]

                        def rope(src_off, dst):
                            s0 = pp[:, src_off:src_off + P]
                            nc.vector.tensor_mul(dst[:], s0, cs)
                            tmp = rp.tile([P, P], bf, tag="ropetmp")
                            nc.vector.tensor_mul(
                                tmp[:, 0:H], pp[:, src_off + H:src_off + P],
                                sn[:, 0:H])
                            nc.vector.tensor_mul(
                                tmp[:, H:P], pp[:, src_off:src_off + H],
                                sn[:, H:P])
                            nc.vector.tensor_add(dst[:], dst[:], tmp[:])

                        rk = rp.tile([P, P], bf, tag="rk")
                        rope(0, rk)
                        tpk = tps.tile([P, P], bf, tag="tp")
                        nc.tensor.transpose(tpk[:], rk[:], identb[:])
                        nc.vector.tensor_copy(kT[:, t128 * P:(t128 + 1) * P],
                                              tpk[:])
                        nc.scalar.copy(vs[:, t128 * VSTR:t128 * VSTR + P],
                                       pp[:, P:2 * P])
                        rq = rp.tile([P, P], bf, tag="rq")
                        rope(2 * P, rq)
                        tpq = tps.tile([P, P], bf, tag="tp")
                        nc.tensor.transpose(tpq[:], rq[:], identb[:])
                        nc.vector.tensor_copy(qT[:, t128 * P:(t128 + 1) * P],
                                              tpq[:])

                    # quarter exchange, all on the otherwise-idle gpsimd
                    # queue. Recovery (ksum/vsum) DMAs for quarter q-1 are
                    # emitted AFTER quarter q's trigger so their CC-waits
                    # never delay the next cin/trigger pair.
                    q4 = tg
                    nc.gpsimd.dma_start(cin[q4][:, 0:4 * P],
                                        kT[:, q4 * 4 * P:(q4 + 1) * 4 * P])
                    vsrc = vs[:, q4 * 4 * VSTR:(q4 + 1) * 4 * VSTR
                              ].rearrange("p (b c) -> p b c", b=4)[:, :, 0:P]
                    nc.gpsimd.dma_start(
                        cin[q4][:, 4 * P:8 * P].rearrange(
                            "p (b c) -> p b c", b=4), vsrc)
                    nc.gpsimd.collective_compute(
                        "AllReduce", mybir.AluOpType.add,
                        replica_groups=[[0, 1], [2, 3], [4, 5], [6, 7]],
                        ins=[cin[q4].opt()], outs=[cout[q4].opt()],
                    )

                    def recover(q):
                        ksum = rec.tile([P, 4 * P], bf, tag="ksum",
                                        name="ksum")
                        vsum = rec.tile([P, 4 * P], bf, tag="vsum",
                                        name="vsum")
                        nc.gpsimd.dma_start(ksum[:], cout[q][:, 0:4 * P])
                        nc.gpsimd.dma_start(vsum[:], cout[q][:, 4 * P:8 * P])
                        nc.vector.tensor_sub(
                            kT[:, (16 + 4 * q) * P:(20 + 4 * q) * P],
                            ksum[:], kT[:, q * 4 * P:(q + 1) * 4 * P])
                        for i in range(4):
                            blk = 16 + 4 * q + i
                            own = 4 * q + i
                            nc.vector.tensor_sub(
                                vs[:, blk * VSTR:blk * VSTR + P],
                                vsum[:, i * P:(i + 1) * P],
                                vs[:, own * VSTR:own * VSTR + P])

                    if tg > 0:
                        recover(tg - 1)
                    if tg == 3:
                        recover(3)

            # ---- phase 2: attention per q-slot ----
            with tc.tile_pool(name="sps", bufs=2, space="PSUM") as sps, \
                 tc.tile_pool(name="ops", bufs=2, space="PSUM") as ops:
                for j in range(4):
                    o = [ops.tile([P, 2 * 129], f32, tag=f"o{m}",
                                  name=f"o{m}") for m in range(2)]
                    nc.vector.memset(o[0][:], 0.0)
                    nc.vector.memset(o[1][:], 0.0)
                    nav = [0] * 4
                    tot = [8 * j + qc + 5 for qc in range(4)]
                    slots = ([("own", si, "full") for si in range(j)]
                             + [("own", j, "diag")]
                             + [("part", pi, "full") for pi in range(j)]
                             + [("part", j, "tail")])
                    for (side, si, kind) in slots:
                        base = si * 4 if side == "own" else 16 + si * 4
                        for sp in range(2):      # pairs (st, st+1)
                            sts = (2 * sp, 2 * sp + 1)
                            Sps = sps.tile([P, 2 * BS], f32, tag="S")
                            Pt = ptp.tile([P, 2 * BS], bf, tag="Pt")
                            offs = []
                            off = 0
                            for st in sts:
                                blk = base + st
                                trim = st * P if kind == "diag" else 0
                                qlen = BS - trim
                                nc.tensor.matmul(
                                    Sps[:, off:off + qlen],
                                    kT[:, blk * P:(blk + 1) * P],
                                    qT[:, j * BS + trim:j * BS + BS],
                                    start=True, stop=True)
                                offs.append((st, blk, trim, qlen, off))
                                off += qlen
                            bias = tb[:, 0:1] if kind == "tail" else 0.0
                            nc.scalar.activation(Pt[:, 0:off], Sps[:, 0:off],
                                                 EXP, bias=bias, scale=SCALE)
                            for (st, blk, trim, qlen, o0) in offs:
                                if kind == "diag":
                                    nc.vector.tensor_mul(
                                        Pt[:, o0:o0 + P], Pt[:, o0:o0 + P],
                                        tri[:])
                                for ch in range(qlen // P):
                                    qc = trim // P + ch
                                    om, oc = o[qc // 2], (qc % 2) * 129
                                    nc.tensor.matmul(
                                        om[:, oc:oc + 129],
                                        Pt[:, o0 + ch * P:o0 + (ch + 1) * P],
                                        vs[:, blk * VSTR:blk * VSTR + 129],
                                        start=False,
                                        stop=(nav[qc] == tot[qc] - 1),
                                        skip_group_check=True)
                                    nav[qc] += 1
                    # normalize + store
                    obig = osb.tile([P, 4 * P], f32, tag="obig")
                    for qc in range(4):
                        om, oc = o[qc // 2], (qc % 2) * 129
                        rcp = osb.tile([P, 1], f32, tag="rcp")
                        nc.vector.reciprocal(rcp[:], om[:, oc + P:oc + P + 1])
                        nc.vector.tensor_scalar_mul(
                            obig[:, qc * P:(qc + 1) * P], om[:, oc:oc + P],
                            rcp[:])
                    nc.sync.dma_start(
                        out[j * BS:(j + 1) * BS, :].rearrange(
                            "(k p) n -> p k n", p=P),
                        obig[:].rearrange("p (k n) -> p k n", k=4))

    bass_rust.generate_event_semaphores(nc)
    return nc


_CACHE = {}


def _get_nc():
    if "nc" not in _CACHE:
        _CACHE["nc"] = build()
    return _CACHE["nc"]


def _prep_inputs(x, Wq, Wk, Wv, cos, sin):
    perm = np.concatenate([np.arange(0, HD, 2), np.arange(1, HD, 2)])
    wq = Wq[perm].astype(np.float32)
    wk = Wk[perm].astype(np.float32)
    w = np.concatenate([wk.T, Wv.T.astype(np.float32), wq.T], axis=1)
    w = np.ascontiguousarray(w).astype(bf16)   # [C, 384] = [k|v|q]
    cos2 = np.concatenate([cos, cos], axis=1).astype(np.float32)
    sin2 = np.concatenate([-sin, sin], axis=1).astype(np.float32)
    in_maps = []
    for c in range(8):
        b, par = c // 2, c % 2
        own = np.concatenate(
            [np.arange(a * BS, (a + 1) * BS) for a in (par, par + 2,
                                                       par + 4, par + 6)])
        xb = np.asarray(x[b], np.float32)
        xtp = np.ascontiguousarray(xb[own].T).astype(bf16)      # [C, T2]
        c2 = np.ascontiguousarray(cos2[own]).astype(bf16)
        s2 = np.ascontiguousarray(sin2[own]).astype(bf16)
        tb = np.full((P, 1), NEG if par == 0 else 0.0, np.float32)
        in_maps.append({"xt": xtp, "w": w, "cos2": c2, "sin2": s2,
                        "tailb": tb})
    return in_maps


def _run(x, Wq, Wk, Wv, cos, sin, trace=False):
    from concourse.bass_utils import run_bass_kernel_spmd
    nc = _get_nc()
    in_maps = _prep_inputs(x, Wq, Wk, Wv, cos, sin)
    res = run_bass_kernel_spmd(nc, in_maps, list(range(8)), trace=trace)
    full = np.empty((B, T, HD), np.float32)
    for c in range(8):
        b, par = c // 2, c % 2
        oc = res.results[c]["out"]
        for j in range(4):
            ab = par + 2 * j
            full[b, ab * BS:(ab + 1) * BS] = oc[j * BS:(j + 1) * BS]
    return full, res


def kernel(x, Wq, Wk, Wv, cos, sin):
    return _run(x, Wq, Wk, Wv, cos, sin, trace=False)[0]
